# revision 83
# baseline (speedup 1.0000x reference)
"""BERT self-attention (B=16, T=512, C=768, H=12, D=64) on 8 trn2 NeuronCores.

Data-parallel over batch: each core gets 2 batches. Matmul operands are fp16
(11-bit mantissa, ~tf32-class precision, 1 cycle/row PE streaming, FWL weight
loads); all accumulation stays fp32 in PSUM. Per core:
  xT    = x transposed on the host during input prep (fp16, [C, M] layout).
  Q^T/K^T ([feature, token] layout, lhsT = W_attn tile) and V ([token, feature]
          layout with per-head [V_h | ones | pad] 128-column blocks so AV
          matmuls keep fast weight loads, lhsT = xT tile).
  S^T   = K^T-as-lhsT matmul -> scores in [key, query] layout (K=64, head pairs
          packed in PE row groups via base-partition-64 slices).
  P     = exp(S/8 + mask) on ScalarE (mask is a per-partition bias in this
          layout), written as fp16.
  y^T   = lhsT=[V_h | ones] matmul -> unnormalized y^T plus softmax row-sums as
          an extra PSUM row; row-sums are collected per batch, inverted with a
          fast Newton-Raphson reciprocal on DVE, replicated across partitions
          by small PE matmuls, and applied with a DVE multiply.
  out   = y^T-as-lhsT matmul with W_proj + b_proj (fp32 result to DRAM).
Bias adds ride the PSUM->SBUF drain (scalar activation / DVE add). Output
stores and staging DMAs round-robin across the three DMA queues.
"""

import sys

sys.path.insert(0, "/opt/trn_rl_repo")

from contextlib import ExitStack

import numpy as np

B, T, C = 16, 512, 768
H, D = 12, 64
C3 = 3 * C
N_CORES = 8
BC = B // N_CORES           # batches per core
M = BC * T                  # tokens per core
KT = C // 128               # feature k-tiles (6)
TT = M // 128               # token tiles per core (8)
NQK = 2 * C // 128          # q+k feature n-tiles (12)
VW = H * 128                # v tile width: per-head [V_h | ones | pad] blocks
SCALE = 1.0 / np.sqrt(D)

_cache = {}


def _build():
    import concourse.bass as bass
    import concourse.tile as tile
    from concourse import bacc, mybir
    f32 = mybir.dt.float32
    f16 = mybir.dt.float16
    Exp = mybir.ActivationFunctionType.Exp
    Ident = mybir.ActivationFunctionType.Identity

    nc = bacc.Bacc("TRN2", target_bir_lowering=False, debug=False,
                   num_devices=N_CORES)
    x_d = nc.dram_tensor("x", [C, M], f16, kind="ExternalInput").ap()
    mask_d = nc.dram_tensor("mask", [BC, T], f32, kind="ExternalInput").ap()
    wa_d = nc.dram_tensor("w_attn", [C, C3], f16, kind="ExternalInput").ap()
    ba_d = nc.dram_tensor("b_attn", [1, C3], f16, kind="ExternalInput").ap()
    wp_d = nc.dram_tensor("w_proj", [C, C], f16, kind="ExternalInput").ap()
    bp_d = nc.dram_tensor("b_proj", [1, C], f16, kind="ExternalInput").ap()
    out_d = nc.dram_tensor("out", [M, C], f32, kind="ExternalOutput").ap()

    # scalar's queue is kept free of bulk DMA: the scalar engine owns the
    # PSUM drains + EXPs and queue entries would delay them (FIFO per engine)
    dmaq2 = [nc.sync, nc.gpsimd]
    dq_state = {"i": 0}

    def next_q():
        q = dmaq2[dq_state["i"] % 2]
        dq_state["i"] += 1
        return q

    with tile.TileContext(nc) as tc, ExitStack() as ctx:
        pp = ctx.enter_context(tc.tile_pool(name="pp", bufs=1))
        np_ = ctx.enter_context(tc.tile_pool(name="norm", bufs=4))
        ap_ = ctx.enter_context(tc.tile_pool(name="att", bufs=6))
        ps_mm = ctx.enter_context(tc.tile_pool(name="ps_mm", bufs=2, space="PSUM"))
        ps_s = ctx.enter_context(tc.tile_pool(name="ps_s", bufs=2, space="PSUM"))
        ps_y = ctx.enter_context(tc.tile_pool(name="ps_y", bufs=2, space="PSUM"))

        # --- prologue -------------------------------------------------
        # Dummy activation first: forces the activation-table const DMA +
        # ACT_TABLE_LOAD to the front of the queues (lazily emitted at first
        # use, it would otherwise sit behind all the bulk input DMAs and
        # stall the first PSUM drains).
        ones = pp.tile([1, 128], f16, tag="ones")
        nc.vector.memset(ones[:], 1.0)
        # sel: block-diagonal one-hot used to broadcast two softmax-recip
        # rows (partitions 2p, 2p+1) to partition halves 0:64 / 64:128
        sel = pp.tile([4, 256], f16, tag="sel")
        nc.vector.memset(sel[:], 0.0)
        for i in range(4):
            nc.gpsimd.dma_start(
                sel[i:i + 1, 64 * i:64 * (i + 1)], ones[0:1, 0:64])

        # DMA schedule: everything the QKV/attention chains consume (wa QK
        # part, xT) goes on the sync+scalar HWDGE queues so the PE never
        # waits on gpsimd, whose SWDGE queue takes multi-us semaphore stalls.
        # gpsimd carries only late-consumed data (V-part of wa, wp, mask,
        # biases).
        wa_t = [pp.tile([128, C3], f16, tag=f"wa{k}", name=f"wa{k}")
                for k in range(KT)]
        # x^T is split per batch so the b=0 chains don't wait on the b=1 half
        xTb = [[pp.tile([128, T], f16, tag=f"xT{b}_{k}", name=f"xT{b}_{k}")
                for k in range(KT)] for b in range(BC)]
        ba_qk = pp.tile([128, NQK], f32, tag="ba_qk")
        nc.gpsimd.dma_start(
            ba_qk[:],
            ba_d[0, 0:2 * C].rearrange("(j p) -> p j", p=128))
        ba_t = pp.tile([1, C3], f16, tag="ba")
        nc.gpsimd.dma_start(ba_t[:], ba_d[:])
        bp_t = pp.tile([1, C], f16, tag="bp")
        nc.gpsimd.dma_start(bp_t[:], bp_d[:])
        # one QK-width W_attn load per k (DMA queues hiccup for ~5us once
        # ~8+ entries deep, so the chain-gating loads stay few and large)
        for k in range(KT):
            q = nc.sync if k % 2 == 0 else nc.scalar
            q.dma_start(xTb[0][k][:], x_d[k * 128:(k + 1) * 128, 0:T])
            q.dma_start(
                wa_t[k][:, 0:1536], wa_d[k * 128:(k + 1) * 128, 0:1536])
        # warm-up activation AFTER the critical DMAs: ACT_TABLE_LOAD then
        # runs behind scalar's queue instead of head-blocking it, and the
        # table const DMA still lands long before the first Ident drain
        warm = pp.tile([1, 2], f32, tag="warm")
        nc.scalar.activation(warm[:], ones[0:1, 0:2], Exp, bias=0.0, scale=1.0)
        for k in range(KT):
            nc.gpsimd.dma_start(
                wa_t[k][:, 1536:2304],
                wa_d[k * 128:(k + 1) * 128, 1536:2304])
        wp_t = [pp.tile([128, C], f16, tag=f"wp{k}", name=f"wp{k}")
                for k in range(KT)]
        for k in range(KT):
            nc.gpsimd.dma_start(wp_t[k][:], wp_d[k * 128:(k + 1) * 128, :])
        for k in range(KT):
            nc.gpsimd.dma_start(
                xTb[1][k][:], x_d[k * 128:(k + 1) * 128, T:M])
        mask_sb = pp.tile([128, BC * 4], f32, tag="mask")
        nc.gpsimd.dma_start(
            mask_sb[:],
            mask_d.rearrange("a b -> (a b)").rearrange("(j p) -> p j", p=128))

        # bias rows replicated across partitions via K=1 matmuls
        ba_v_rep = pp.tile([128, C], f32, tag="ba_v_rep")
        bp_rep = pp.tile([128, C], f32, tag="bp_rep")

        def brep_emit():
            # uses ps_y (idle until attention) so the chains' ps_mm rotation
            # is not coupled to these drains
            for lo, w in ((0, 512), (512, 256)):
                p = ps_y.tile([128, 512], f32, tag="py", name=f"brep{lo}")
                nc.tensor.matmul(
                    p[:, :w], ones[0:1, 0:128],
                    ba_t[0:1, 2 * C + lo:2 * C + lo + w], start=True, stop=True)
                nc.vector.tensor_copy(ba_v_rep[:, lo:lo + w], p[:, :w])
                p2 = ps_y.tile([128, 512], f32, tag="py", name=f"bprep{lo}")
                nc.tensor.matmul(
                    p2[:, :w], ones[0:1, 0:128],
                    bp_t[0:1, lo:lo + w], start=True, stop=True)
                nc.vector.tensor_copy(bp_rep[:, lo:lo + w], p2[:, :w])

        v_t = [pp.tile([128, VW], f16, tag=f"v{t}", name=f"v{t}")
               for t in range(TT)]
        qkT = [pp.tile([128, M], f16, tag=f"qk{n}", name=f"qk{n}")
               for n in range(NQK)]
        yT_t = [pp.tile([128, M], f16, tag=f"yT{c}", name=f"yT{c}")
                for c in range(KT)]
        def v_memset_emit(ts):
            # b=0's tiles are set while the DVE is idle pre-chain; b=1's are
            # deferred into phase 1 so they don't pace the b=0 V drains
            for t in ts:
                nc.vector.memset(
                    v_t[t].rearrange("p (h c) -> p h c", c=128)
                    [:, :, D:128], 1.0)

        def qkv_chain(b, i):
            """i in [0, 20): 12 QK n-tiles then 8 V half-tiles."""
            bcol = b * T
            xT = xTb[b]
            if i < NQK:
                n = i
                p = ps_mm.tile([128, 512], f32, tag="mm", name=f"mm{b}_{i}")
                for k in range(KT):
                    nc.tensor.matmul(
                        p[:],
                        wa_t[k][:, n * 128:(n + 1) * 128],
                        xT[k][:, 0:T],
                        start=(k == 0), stop=(k == KT - 1))
                # drain PSUM->SBUF fused with bias add; split scalar/DVE.
                # b=0's first chains drain on DVE: the scalar engine is still
                # working through its prologue DMA queue then.
                if n % 2 == 0 and not (b == 0 and n < 6):
                    nc.scalar.activation(
                        qkT[n][:, bcol:bcol + T], p[:], Ident,
                        bias=ba_qk[:, n:n + 1], scale=1.0)
                else:
                    nc.vector.tensor_scalar_add(
                        qkT[n][:, bcol:bcol + T], p[:], ba_qk[:, n:n + 1])
            else:
                j = i - NQK
                t = b * 4 + j // 2
                lo, w = ((0, 512), (512, 256))[j % 2]
                p = ps_mm.tile([128, 512], f32, tag="mm", name=f"mm{b}_{i}")
                tl = (t - b * 4) * 128
                for k in range(KT):
                    nc.tensor.matmul(
                        p[:, :w],
                        xT[k][:, tl:tl + 128],
                        wa_t[k][:, 2 * C + lo:2 * C + lo + w],
                        start=(k == 0), stop=(k == KT - 1))
                h0 = lo // D
                nc.vector.tensor_tensor(
                    out=v_t[t].rearrange("p (h c) -> p h c", c=128)
                        [:, h0:h0 + w // D, 0:D],
                    in0=p[:, :w].rearrange("p (h c) -> p h c", c=D),
                    in1=ba_v_rep[:, lo:lo + w].rearrange(
                        "p (h c) -> p h c", c=D),
                    op=mybir.AluOpType.add)

        yun_all = {}
        r_tiles = {}

        def attention_hp(b, hp):
            bcol = b * T
            if hp % 2 == 0:
                rt = np_.tile([4, 512], f32, tag="r_all", bufs=3,
                              name=f"r_all{b}_{hp // 2}")
                nc.vector.memset(rt[:], 1.0)
                r_tiles[(b, hp // 2)] = rt
            e_tiles = []
            for kt in range(4):
                ps = ps_s.tile([128, 1024], f32)
                for sub in range(2):
                    r0 = 64 * sub
                    nc.tensor.matmul(
                        ps[:, sub * 512:sub * 512 + 512],
                        qkT[6 + hp][r0:r0 + D,
                                    bcol + kt * 128:bcol + (kt + 1) * 128],
                        qkT[hp][r0:r0 + D, bcol:bcol + T],
                        start=True, stop=True)
                e = ap_.tile([128, 1024], f16, tag="e")
                nc.scalar.activation(
                    e[:], ps[:], Exp,
                    bias=mask_sb[:, b * 4 + kt:b * 4 + kt + 1],
                    scale=float(SCALE))
                e_tiles.append(e)
            # both heads' unnormalized y land in one [128, 512] pair tile
            # (halves written via per-operand partition bases) so the later
            # normalize is a single full-height DVE multiply per pair
            yun = np_.tile([128, 512], f16, tag="yun", bufs=7,
                           name=f"yun{b}_{hp}")
            for sub in range(2):
                h = 2 * hp + sub
                py = ps_y.tile([128, 512], f32)
                for kt in range(4):
                    nc.tensor.matmul(
                        py[:, :],
                        v_t[b * 4 + kt][:, 128 * h:128 * (h + 1)],
                        e_tiles[kt][:, sub * 512:sub * 512 + 512],
                        start=(kt == 0), stop=(kt == 3))
                nc.vector.tensor_copy(
                    yun[64 * sub:64 * sub + D, :], py[0:D, :])
                rs = np_.tile([D + 1, 512], f32, tag="rstage")
                nc.vector.tensor_copy(rs[D:D + 1, :], py[D:D + 1, :])
                next_q().dma_start(
                    r_tiles[(b, hp // 2)][h % 4:h % 4 + 1, :],
                    rs[D:D + 1, :])
            yun_all[(b, hp)] = yun

        recip_tiles = {}

        def norm_recip(b, grp):
            recip = np_.tile([4, 512], f32, tag="recip", bufs=3)
            nc.vector.reciprocal_approx_fast(recip[:], r_tiles[(b, grp)][:])
            recip16 = np_.tile([4, 512], f16, tag="recip16", bufs=3)
            nc.vector.tensor_copy(recip16[:], recip[:])
            recip_tiles[(b, grp)] = recip16

        def norm_apply(b, grp):
            bcol = b * T
            recip16 = recip_tiles[(b, grp)]
            # one K=4 matmul broadcasts a head-pair's recip rows to partition
            # halves 0:64 / 64:128; DVE multiplies write each half of the
            # shared yT tile directly (partition bases may differ per operand)
            for p2 in range(2):
                hA = 4 * grp + 2 * p2
                nt = hA // 2
                rep = ps_y.tile([128, 512], f32, tag="py",
                                name=f"rep{b}_{hA}")
                nc.tensor.matmul(
                    rep[:, :], sel[0:4, 128 * p2:128 * (p2 + 1)],
                    recip16[0:4, :], start=True, stop=True)
                nc.vector.tensor_mul(
                    yT_t[nt][0:2 * D, bcol:bcol + T],
                    yun_all[(b, nt)][:], rep[:, :])

        pj_part = {}
        fin_state = {"i": 0}

        def proj_chunk(b, i, ks=0, ke=KT, partial=False, final=False):
            t = b * 4 + i // 2
            lo, w = ((0, 512), (512, 256))[i % 2]
            p = ps_mm.tile([128, 512], f32, tag="mm", name=f"pj{b}_{i}_{ks}")
            for k in range(ks, ke):
                nc.tensor.matmul(
                    p[:, :w],
                    yT_t[k][:, t * 128:(t + 1) * 128],
                    wp_t[k][:, lo:lo + w],
                    start=(k == ks), stop=(k == ke - 1))
            if partial:
                pt = np_.tile([128, 512], f32, tag="pjpart", bufs=8,
                              name=f"pjpart{i}")
                nc.vector.tensor_tensor(
                    out=pt[:, :w], in0=p[:, :w], in1=bp_rep[:, lo:lo + w],
                    op=mybir.AluOpType.add)
                pj_part[(b, i)] = pt
                return
            ot = np_.tile([128, 512], f32, tag="ostage", bufs=3)
            if (b, i) in pj_part:
                nc.vector.tensor_tensor(
                    out=ot[:, :w], in0=p[:, :w], in1=pj_part[(b, i)][:, :w],
                    op=mybir.AluOpType.add)
            else:
                nc.vector.tensor_tensor(
                    out=ot[:, :w], in0=p[:, :w], in1=bp_rep[:, lo:lo + w],
                    op=mybir.AluOpType.add)
            if final:
                q = [nc.scalar, nc.sync][fin_state["i"] % 2]
                fin_state["i"] += 1
            else:
                q = next_q()
            q.dma_start(out_d[t * 128:(t + 1) * 128, lo:lo + w], ot[:, :w])

        # software-pipelined emission
        CHAIN_ORDER = list(range(9)) + list(range(12, 20)) + [9, 10, 11]
        v_memset_emit(range(4))
        # brep after six chains: its ba/bp deps sit early on gpsimd's long
        # queue, whose coarse completion-wait only resolves around 20us --
        # emitted earlier it head-blocks the in-order PE stream
        for i in CHAIN_ORDER[:6]:
            qkv_chain(0, i)
        brep_emit()
        for i in CHAIN_ORDER[6:]:
            qkv_chain(0, i)
        qk1 = iter(CHAIN_ORDER)
        for hp in range(6):
            # apply for the previous hp-pair goes BEFORE this attention_hp:
            # its rep matmuls then use ps_y tiles already drained, instead of
            # head-blocking the PE queue on this hp's PSUM rotation
            if hp % 2 == 1 and hp >= 3:
                norm_apply(0, hp // 2 - 1)
            attention_hp(0, hp)
            if hp == 1:
                v_memset_emit(range(4, TT))
            if hp % 2 == 1:
                norm_recip(0, hp // 2)
            for _ in range(4 if hp < 2 else 3):
                i = next(qk1, None)
                if i is not None:
                    qkv_chain(1, i)
        norm_apply(0, 2)
        pj0 = iter(range(8))
        for hp in range(6):
            if hp % 2 == 1 and hp >= 3:
                norm_apply(1, hp // 2 - 1)
            attention_hp(1, hp)
            if hp % 2 == 1:
                norm_recip(1, hp // 2)
            if hp == 5:
                for i in range(8):
                    proj_chunk(1, i, 0, 4, partial=True)
            i = next(pj0, None)
            if i is not None:
                proj_chunk(0, i)
        norm_apply(1, 2)
        for i in pj0:
            proj_chunk(0, i)
        # final b=1 proj: attention is over, so ps_y and ps_mm alternate for
        # an effective 4-deep rotation decoupling matmuls from DVE drains;
        # one full-width store per token tile trims end-of-program latency
        for t4 in range(4, 8):
            ot = np_.tile([128, C], f32, tag="ofin", bufs=2, name=f"ofin{t4}")
            for j, (lo, w) in enumerate(((0, 512), (512, 256))):
                i = 2 * (t4 - 4) + j
                pool = ps_mm if (t4 + j) % 2 == 0 else ps_y
                tagn = "mm" if (t4 + j) % 2 == 0 else "py"
                p = pool.tile([128, 512], f32, tag=tagn,
                              name=f"pjf{t4}_{j}")
                for k in range(4, KT):
                    nc.tensor.matmul(
                        p[:, :w],
                        yT_t[k][:, t4 * 128:(t4 + 1) * 128],
                        wp_t[k][:, lo:lo + w],
                        start=(k == 4), stop=(k == KT - 1))
                nc.vector.tensor_tensor(
                    out=ot[:, lo:lo + w], in0=p[:, :w],
                    in1=pj_part[(1, i)][:, :w], op=mybir.AluOpType.add)
            q = [nc.scalar, nc.sync][t4 % 2]
            q.dma_start(out_d[t4 * 128:(t4 + 1) * 128, :], ot[:, :])

    nc.compile()
    return nc


def get_compiled():
    if "nc" not in _cache:
        _cache["nc"] = _build()
    return _cache["nc"]


def make_in_maps(x, attention_mask, W_attn, b_attn, W_proj, b_proj):
    x = np.asarray(x, dtype=np.float32).astype(np.float16)
    mask = np.ascontiguousarray(
        np.asarray(attention_mask, dtype=np.float32)[:, 0, 0, :])
    wa = np.asarray(W_attn, dtype=np.float32).astype(np.float16)
    ba = np.asarray(b_attn, dtype=np.float32).astype(np.float16).reshape(1, C3)
    wp = np.asarray(W_proj, dtype=np.float32).astype(np.float16)
    bp = np.asarray(b_proj, dtype=np.float32).astype(np.float16).reshape(1, C)
    maps = []
    for i in range(N_CORES):
        maps.append({
            "x": np.ascontiguousarray(x[BC * i:BC * (i + 1)].reshape(M, C).T),
            "mask": np.ascontiguousarray(mask[BC * i:BC * (i + 1)]),
            "w_attn": wa, "b_attn": ba, "w_proj": wp, "b_proj": bp,
        })
    return maps


def kernel(x, attention_mask, W_attn, b_attn, W_proj, b_proj):
    from concourse.bass_utils import run_bass_kernel_spmd

    nc = get_compiled()
    in_maps = make_in_maps(x, attention_mask, W_attn, b_attn, W_proj, b_proj)
    last_err = None
    for _ in range(3):
        try:
            res = run_bass_kernel_spmd(nc, in_maps, list(range(N_CORES)))
            break
        except Exception as e:  # transient NRT device errors: retry
            last_err = e
    else:
        raise last_err
    out = np.concatenate(
        [res.results[i]["out"].reshape(BC, T, C) for i in range(N_CORES)], axis=0)
    return out.astype(np.float32)


# revision 85
# speedup vs baseline: 1.0191x; 1.0191x over previous
"""BERT self-attention (B=16, T=512, C=768, H=12, D=64) on 8 trn2 NeuronCores.

Data-parallel over batch: each core gets 2 batches. Matmul operands are fp16
(11-bit mantissa, ~tf32-class precision, 1 cycle/row PE streaming, FWL weight
loads); all accumulation stays fp32 in PSUM. Per core:
  xT    = x transposed on the host during input prep (fp16, [C, M] layout).
  Q^T/K^T ([feature, token] layout, lhsT = W_attn tile) and V ([token, feature]
          layout with per-head [V_h | ones | pad] 128-column blocks so AV
          matmuls keep fast weight loads, lhsT = xT tile).
  S^T   = K^T-as-lhsT matmul -> scores in [key, query] layout (K=64, head pairs
          packed in PE row groups via base-partition-64 slices).
  P     = exp(S/8 + mask) on ScalarE (mask is a per-partition bias in this
          layout), written as fp16.
  y^T   = lhsT=[V_h | ones] matmul -> unnormalized y^T plus softmax row-sums as
          an extra PSUM row; row-sums are collected per batch, inverted with a
          fast Newton-Raphson reciprocal on DVE, replicated across partitions
          by small PE matmuls, and applied with a DVE multiply.
  out   = y^T-as-lhsT matmul with W_proj + b_proj (fp32 result to DRAM).
Bias adds ride the PSUM->SBUF drain (scalar activation / DVE add). Output
stores and staging DMAs round-robin across the three DMA queues.
"""

import sys

sys.path.insert(0, "/opt/trn_rl_repo")

from contextlib import ExitStack

import numpy as np

B, T, C = 16, 512, 768
H, D = 12, 64
C3 = 3 * C
N_CORES = 8
BC = B // N_CORES           # batches per core
M = BC * T                  # tokens per core
KT = C // 128               # feature k-tiles (6)
TT = M // 128               # token tiles per core (8)
NQK = 2 * C // 128          # q+k feature n-tiles (12)
VW = H * 128                # v tile width: per-head [V_h | ones | pad] blocks
SCALE = 1.0 / np.sqrt(D)

_cache = {}


def _build():
    import concourse.bass as bass
    import concourse.tile as tile
    from concourse import bacc, mybir
    f32 = mybir.dt.float32
    f16 = mybir.dt.float16
    Exp = mybir.ActivationFunctionType.Exp
    Ident = mybir.ActivationFunctionType.Identity

    nc = bacc.Bacc("TRN2", target_bir_lowering=False, debug=False,
                   num_devices=N_CORES)
    x_d = nc.dram_tensor("x", [C, M], f16, kind="ExternalInput").ap()
    mask_d = nc.dram_tensor("mask", [BC, T], f32, kind="ExternalInput").ap()
    wa_d = nc.dram_tensor("w_attn", [C, C3], f16, kind="ExternalInput").ap()
    ba_d = nc.dram_tensor("b_attn", [1, C3], f16, kind="ExternalInput").ap()
    wp_d = nc.dram_tensor("w_proj", [C, C], f16, kind="ExternalInput").ap()
    bp_d = nc.dram_tensor("b_proj", [1, C], f16, kind="ExternalInput").ap()
    out_d = nc.dram_tensor("out", [M, C], f32, kind="ExternalOutput").ap()

    # scalar's queue is kept free of bulk DMA: the scalar engine owns the
    # PSUM drains + EXPs and queue entries would delay them (FIFO per engine)
    dmaq2 = [nc.sync, nc.gpsimd]
    dq_state = {"i": 0}

    def next_q():
        q = dmaq2[dq_state["i"] % 2]
        dq_state["i"] += 1
        return q

    with tile.TileContext(nc) as tc, ExitStack() as ctx:
        pp = ctx.enter_context(tc.tile_pool(name="pp", bufs=1))
        np_ = ctx.enter_context(tc.tile_pool(name="norm", bufs=4))
        ap_ = ctx.enter_context(tc.tile_pool(name="att", bufs=6))
        ps_mm = ctx.enter_context(tc.tile_pool(name="ps_mm", bufs=2, space="PSUM"))
        ps_s = ctx.enter_context(tc.tile_pool(name="ps_s", bufs=2, space="PSUM"))
        ps_y = ctx.enter_context(tc.tile_pool(name="ps_y", bufs=2, space="PSUM"))

        # --- prologue -------------------------------------------------
        # Dummy activation first: forces the activation-table const DMA +
        # ACT_TABLE_LOAD to the front of the queues (lazily emitted at first
        # use, it would otherwise sit behind all the bulk input DMAs and
        # stall the first PSUM drains).
        ones = pp.tile([1, 128], f16, tag="ones")
        nc.vector.memset(ones[:], 1.0)
        # sel: block-diagonal one-hot used to broadcast two softmax-recip
        # rows (partitions 2p, 2p+1) to partition halves 0:64 / 64:128
        sel = pp.tile([4, 256], f16, tag="sel")
        nc.vector.memset(sel[:], 0.0)
        for i in range(4):
            nc.gpsimd.dma_start(
                sel[i:i + 1, 64 * i:64 * (i + 1)], ones[0:1, 0:64])

        # DMA schedule: everything the QKV/attention chains consume (wa QK
        # part, xT) goes on the sync+scalar HWDGE queues so the PE never
        # waits on gpsimd, whose SWDGE queue takes multi-us semaphore stalls.
        # gpsimd carries only late-consumed data (V-part of wa, wp, mask,
        # biases).
        wa_t = [pp.tile([128, C3], f16, tag=f"wa{k}", name=f"wa{k}")
                for k in range(KT)]
        # x^T is split per batch so the b=0 chains don't wait on the b=1 half
        xTb = [[pp.tile([128, T], f16, tag=f"xT{b}_{k}", name=f"xT{b}_{k}")
                for k in range(KT)] for b in range(BC)]
        ba_qk = pp.tile([128, NQK], f32, tag="ba_qk")
        nc.gpsimd.dma_start(
            ba_qk[:],
            ba_d[0, 0:2 * C].rearrange("(j p) -> p j", p=128))
        ba_t = pp.tile([1, C3], f16, tag="ba")
        nc.gpsimd.dma_start(ba_t[:], ba_d[:])
        bp_t = pp.tile([1, C], f16, tag="bp")
        nc.gpsimd.dma_start(bp_t[:], bp_d[:])
        # one QK-width W_attn load per k (DMA queues hiccup for ~5us once
        # ~8+ entries deep, so the chain-gating loads stay few and large)
        for k in range(KT):
            q = nc.sync if k % 2 == 0 else nc.scalar
            q.dma_start(xTb[0][k][:], x_d[k * 128:(k + 1) * 128, 0:T])
            q.dma_start(
                wa_t[k][:, 0:1536], wa_d[k * 128:(k + 1) * 128, 0:1536])
        # warm-up activation AFTER the critical DMAs: ACT_TABLE_LOAD then
        # runs behind scalar's queue instead of head-blocking it, and the
        # table const DMA still lands long before the first Ident drain
        warm = pp.tile([1, 2], f32, tag="warm")
        nc.scalar.activation(warm[:], ones[0:1, 0:2], Exp, bias=0.0, scale=1.0)
        for k in range(KT):
            nc.gpsimd.dma_start(
                wa_t[k][:, 1536:2304],
                wa_d[k * 128:(k + 1) * 128, 1536:2304])
        wp_t = [pp.tile([128, C], f16, tag=f"wp{k}", name=f"wp{k}")
                for k in range(KT)]
        for k in range(KT):
            nc.gpsimd.dma_start(wp_t[k][:], wp_d[k * 128:(k + 1) * 128, :])
        for k in range(KT):
            nc.gpsimd.dma_start(
                xTb[1][k][:], x_d[k * 128:(k + 1) * 128, T:M])
        mask_sb = pp.tile([128, BC * 4], f32, tag="mask")
        nc.gpsimd.dma_start(
            mask_sb[:],
            mask_d.rearrange("a b -> (a b)").rearrange("(j p) -> p j", p=128))

        # bias rows replicated across partitions via K=1 matmuls
        ba_v_rep = pp.tile([128, C], f32, tag="ba_v_rep")
        bp_rep = pp.tile([128, C], f32, tag="bp_rep")

        def brep_emit():
            # uses ps_y (idle until attention) so the chains' ps_mm rotation
            # is not coupled to these drains
            for lo, w in ((0, 512), (512, 256)):
                p = ps_y.tile([128, 512], f32, tag="py", name=f"brep{lo}")
                nc.tensor.matmul(
                    p[:, :w], ones[0:1, 0:128],
                    ba_t[0:1, 2 * C + lo:2 * C + lo + w], start=True, stop=True)
                nc.vector.tensor_copy(ba_v_rep[:, lo:lo + w], p[:, :w])
                p2 = ps_y.tile([128, 512], f32, tag="py", name=f"bprep{lo}")
                nc.tensor.matmul(
                    p2[:, :w], ones[0:1, 0:128],
                    bp_t[0:1, lo:lo + w], start=True, stop=True)
                nc.vector.tensor_copy(bp_rep[:, lo:lo + w], p2[:, :w])

        v_t = [pp.tile([128, VW], f16, tag=f"v{t}", name=f"v{t}")
               for t in range(TT)]
        qkT = [pp.tile([128, M], f16, tag=f"qk{n}", name=f"qk{n}")
               for n in range(NQK)]
        yT_t = [pp.tile([128, M], f16, tag=f"yT{c}", name=f"yT{c}")
                for c in range(KT)]
        def v_memset_emit(ts):
            # b=0's tiles are set while the DVE is idle pre-chain; b=1's are
            # deferred into phase 1 so they don't pace the b=0 V drains
            for t in ts:
                nc.vector.memset(
                    v_t[t].rearrange("p (h c) -> p h c", c=128)
                    [:, :, D:128], 1.0)

        def qkv_chain(b, i):
            """i in [0, 20): 12 QK n-tiles then 8 V half-tiles."""
            bcol = b * T
            xT = xTb[b]
            if i < NQK:
                n = i
                p = ps_mm.tile([128, 512], f32, tag="mm", name=f"mm{b}_{i}")
                for k in range(KT):
                    nc.tensor.matmul(
                        p[:],
                        wa_t[k][:, n * 128:(n + 1) * 128],
                        xT[k][:, 0:T],
                        start=(k == 0), stop=(k == KT - 1))
                # drain PSUM->SBUF fused with bias add; split scalar/DVE.
                # b=0's first chains drain on DVE: the scalar engine is still
                # working through its prologue DMA queue then.
                if n % 2 == 0 and not (b == 0 and n < 6):
                    nc.scalar.activation(
                        qkT[n][:, bcol:bcol + T], p[:], Ident,
                        bias=ba_qk[:, n:n + 1], scale=1.0)
                else:
                    nc.vector.tensor_scalar_add(
                        qkT[n][:, bcol:bcol + T], p[:], ba_qk[:, n:n + 1])
            else:
                j = i - NQK
                t = b * 4 + j // 2
                lo, w = ((0, 512), (512, 256))[j % 2]
                p = ps_mm.tile([128, 512], f32, tag="mm", name=f"mm{b}_{i}")
                tl = (t - b * 4) * 128
                for k in range(KT):
                    nc.tensor.matmul(
                        p[:, :w],
                        xT[k][:, tl:tl + 128],
                        wa_t[k][:, 2 * C + lo:2 * C + lo + w],
                        start=(k == 0), stop=(k == KT - 1))
                h0 = lo // D
                nc.vector.tensor_tensor(
                    out=v_t[t].rearrange("p (h c) -> p h c", c=128)
                        [:, h0:h0 + w // D, 0:D],
                    in0=p[:, :w].rearrange("p (h c) -> p h c", c=D),
                    in1=ba_v_rep[:, lo:lo + w].rearrange(
                        "p (h c) -> p h c", c=D),
                    op=mybir.AluOpType.add)

        yun_all = {}
        r_tiles = {}

        def attention_hp(b, hp):
            bcol = b * T
            if hp % 2 == 0:
                rt = np_.tile([4, 512], f32, tag="r_all", bufs=3,
                              name=f"r_all{b}_{hp // 2}")
                nc.vector.memset(rt[:], 1.0)
                r_tiles[(b, hp // 2)] = rt
            e_tiles = []
            for kt in range(4):
                ps = ps_s.tile([128, 1024], f32)
                for sub in range(2):
                    r0 = 64 * sub
                    nc.tensor.matmul(
                        ps[:, sub * 512:sub * 512 + 512],
                        qkT[6 + hp][r0:r0 + D,
                                    bcol + kt * 128:bcol + (kt + 1) * 128],
                        qkT[hp][r0:r0 + D, bcol:bcol + T],
                        start=True, stop=True)
                e = ap_.tile([128, 1024], f16, tag="e")
                nc.scalar.activation(
                    e[:], ps[:], Exp,
                    bias=mask_sb[:, b * 4 + kt:b * 4 + kt + 1],
                    scale=float(SCALE))
                e_tiles.append(e)
            # both heads' unnormalized y land in one [128, 512] pair tile
            # (halves written via per-operand partition bases) so the later
            # normalize is a single full-height DVE multiply per pair
            yun = np_.tile([128, 512], f16, tag="yun", bufs=7,
                           name=f"yun{b}_{hp}")
            for sub in range(2):
                h = 2 * hp + sub
                py = ps_y.tile([128, 512], f32)
                for kt in range(4):
                    nc.tensor.matmul(
                        py[:, :],
                        v_t[b * 4 + kt][:, 128 * h:128 * (h + 1)],
                        e_tiles[kt][:, sub * 512:sub * 512 + 512],
                        start=(kt == 0), stop=(kt == 3))
                nc.vector.tensor_copy(
                    yun[64 * sub:64 * sub + D, :], py[0:D, :])
                rs = np_.tile([D + 1, 512], f32, tag="rstage")
                nc.vector.tensor_copy(rs[D:D + 1, :], py[D:D + 1, :])
                next_q().dma_start(
                    r_tiles[(b, hp // 2)][h % 4:h % 4 + 1, :],
                    rs[D:D + 1, :])
            yun_all[(b, hp)] = yun

        recip_tiles = {}

        def norm_recip(b, grp):
            recip = np_.tile([4, 512], f32, tag="recip", bufs=3)
            nc.vector.reciprocal_approx_fast(recip[:], r_tiles[(b, grp)][:])
            recip16 = np_.tile([4, 512], f16, tag="recip16", bufs=3)
            nc.vector.tensor_copy(recip16[:], recip[:])
            recip_tiles[(b, grp)] = recip16

        def norm_apply(b, grp):
            bcol = b * T
            recip16 = recip_tiles[(b, grp)]
            # one K=4 matmul broadcasts a head-pair's recip rows to partition
            # halves 0:64 / 64:128; DVE multiplies write each half of the
            # shared yT tile directly (partition bases may differ per operand)
            for p2 in range(2):
                hA = 4 * grp + 2 * p2
                nt = hA // 2
                rep = ps_y.tile([128, 512], f32, tag="py",
                                name=f"rep{b}_{hA}")
                nc.tensor.matmul(
                    rep[:, :], sel[0:4, 128 * p2:128 * (p2 + 1)],
                    recip16[0:4, :], start=True, stop=True)
                nc.vector.tensor_mul(
                    yT_t[nt][0:2 * D, bcol:bcol + T],
                    yun_all[(b, nt)][:], rep[:, :])

        pj_part = {}
        fin_state = {"i": 0}

        def proj_chunk(b, i, ks=0, ke=KT, partial=False, final=False):
            t = b * 4 + i // 2
            lo, w = ((0, 512), (512, 256))[i % 2]
            p = ps_mm.tile([128, 512], f32, tag="mm", name=f"pj{b}_{i}_{ks}")
            for k in range(ks, ke):
                nc.tensor.matmul(
                    p[:, :w],
                    yT_t[k][:, t * 128:(t + 1) * 128],
                    wp_t[k][:, lo:lo + w],
                    start=(k == ks), stop=(k == ke - 1))
            if partial:
                pt = np_.tile([128, 512], f32, tag="pjpart", bufs=8,
                              name=f"pjpart{i}")
                nc.vector.tensor_tensor(
                    out=pt[:, :w], in0=p[:, :w], in1=bp_rep[:, lo:lo + w],
                    op=mybir.AluOpType.add)
                pj_part[(b, i)] = pt
                return
            ot = np_.tile([128, 512], f32, tag="ostage", bufs=3)
            if (b, i) in pj_part:
                nc.vector.tensor_tensor(
                    out=ot[:, :w], in0=p[:, :w], in1=pj_part[(b, i)][:, :w],
                    op=mybir.AluOpType.add)
            else:
                nc.vector.tensor_tensor(
                    out=ot[:, :w], in0=p[:, :w], in1=bp_rep[:, lo:lo + w],
                    op=mybir.AluOpType.add)
            if final:
                q = [nc.scalar, nc.sync][fin_state["i"] % 2]
                fin_state["i"] += 1
            else:
                q = next_q()
            q.dma_start(out_d[t * 128:(t + 1) * 128, lo:lo + w], ot[:, :w])

        # software-pipelined emission
        CHAIN_ORDER = list(range(9)) + list(range(12, 20)) + [9, 10, 11]
        v_memset_emit(range(4))
        # brep after six chains: its ba/bp deps sit early on gpsimd's long
        # queue, whose coarse completion-wait only resolves around 20us --
        # emitted earlier it head-blocks the in-order PE stream
        for i in CHAIN_ORDER[:6]:
            qkv_chain(0, i)
        brep_emit()
        for i in CHAIN_ORDER[6:]:
            qkv_chain(0, i)
        qk1 = iter(CHAIN_ORDER)
        for hp in range(6):
            # apply for the previous hp-pair goes BEFORE this attention_hp:
            # its rep matmuls then use ps_y tiles already drained, instead of
            # head-blocking the PE queue on this hp's PSUM rotation
            if hp % 2 == 1 and hp >= 3:
                norm_apply(0, hp // 2 - 1)
            attention_hp(0, hp)
            if hp == 1:
                v_memset_emit(range(4, TT))
            if hp % 2 == 1:
                norm_recip(0, hp // 2)
            # one chain is held back as slack work for the phase boundary,
            # where the PE otherwise idles on apply(0,2)'s multiplies
            for _ in range((4, 4, 3, 3, 3, 2)[hp]):
                i = next(qk1, None)
                if i is not None:
                    qkv_chain(1, i)
        norm_apply(0, 2)
        pj0 = iter(range(8))
        for hp in range(6):
            if hp % 2 == 1 and hp >= 3:
                norm_apply(1, hp // 2 - 1)
            attention_hp(1, hp)
            if hp == 0:
                for i in qk1:
                    qkv_chain(1, i)
            if hp % 2 == 1:
                norm_recip(1, hp // 2)
            if hp == 5:
                for i in range(8):
                    proj_chunk(1, i, 0, 4, partial=True)
            i = next(pj0, None)
            if i is not None:
                proj_chunk(0, i)
        norm_apply(1, 2)
        for i in pj0:
            proj_chunk(0, i)
        # final b=1 proj: attention is over, so ps_y and ps_mm alternate for
        # an effective 4-deep rotation decoupling matmuls from DVE drains;
        # one full-width store per token tile trims end-of-program latency
        for t4 in range(4, 8):
            ot = np_.tile([128, C], f32, tag="ofin", bufs=2, name=f"ofin{t4}")
            for j, (lo, w) in enumerate(((0, 512), (512, 256))):
                i = 2 * (t4 - 4) + j
                pool = ps_mm if (t4 + j) % 2 == 0 else ps_y
                tagn = "mm" if (t4 + j) % 2 == 0 else "py"
                p = pool.tile([128, 512], f32, tag=tagn,
                              name=f"pjf{t4}_{j}")
                for k in range(4, KT):
                    nc.tensor.matmul(
                        p[:, :w],
                        yT_t[k][:, t4 * 128:(t4 + 1) * 128],
                        wp_t[k][:, lo:lo + w],
                        start=(k == 4), stop=(k == KT - 1))
                nc.vector.tensor_tensor(
                    out=ot[:, lo:lo + w], in0=p[:, :w],
                    in1=pj_part[(1, i)][:, :w], op=mybir.AluOpType.add)
            q = [nc.scalar, nc.sync][t4 % 2]
            q.dma_start(out_d[t4 * 128:(t4 + 1) * 128, :], ot[:, :])

    nc.compile()
    return nc


def get_compiled():
    if "nc" not in _cache:
        _cache["nc"] = _build()
    return _cache["nc"]


def make_in_maps(x, attention_mask, W_attn, b_attn, W_proj, b_proj):
    x = np.asarray(x, dtype=np.float32).astype(np.float16)
    mask = np.ascontiguousarray(
        np.asarray(attention_mask, dtype=np.float32)[:, 0, 0, :])
    wa = np.asarray(W_attn, dtype=np.float32).astype(np.float16)
    ba = np.asarray(b_attn, dtype=np.float32).astype(np.float16).reshape(1, C3)
    wp = np.asarray(W_proj, dtype=np.float32).astype(np.float16)
    bp = np.asarray(b_proj, dtype=np.float32).astype(np.float16).reshape(1, C)
    maps = []
    for i in range(N_CORES):
        maps.append({
            "x": np.ascontiguousarray(x[BC * i:BC * (i + 1)].reshape(M, C).T),
            "mask": np.ascontiguousarray(mask[BC * i:BC * (i + 1)]),
            "w_attn": wa, "b_attn": ba, "w_proj": wp, "b_proj": bp,
        })
    return maps


def kernel(x, attention_mask, W_attn, b_attn, W_proj, b_proj):
    from concourse.bass_utils import run_bass_kernel_spmd

    nc = get_compiled()
    in_maps = make_in_maps(x, attention_mask, W_attn, b_attn, W_proj, b_proj)
    last_err = None
    for _ in range(3):
        try:
            res = run_bass_kernel_spmd(nc, in_maps, list(range(N_CORES)))
            break
        except Exception as e:  # transient NRT device errors: retry
            last_err = e
    else:
        raise last_err
    out = np.concatenate(
        [res.results[i]["out"].reshape(BC, T, C) for i in range(N_CORES)], axis=0)
    return out.astype(np.float32)
